# revision 8
# baseline (speedup 1.0000x reference)
"""Trainium2 Bass kernel for nn_CriticEncoder (2-layer LSTM + causal MHA attn-gate).

Strategy: data-parallel over batch across 8 cores (b=4 per core), everything
core-local. Per core:
  P1: gates0_in.T = Wih0r @ x.T   (big weight-stationary matmul -> DRAM stream)
  P2: L0 recurrence, weight-stationary gates.T = Whh0r @ h.T; [hidden,batch]
      layout throughout, h history kept in SBUF (bf16)
  P3: gates1_in.T = Wih1r @ h0.T  (big matmul from SBUF-resident h0)
  P4: L1 recurrence -> h1 history in SBUF
  P5: fused attention: qT/kT projections, per-(sample,head-pair) scores on PE,
      exp on ACT (scores are tiny -> no max subtraction), causal mask by
      block structure + tril on diagonal blocks, numer/denom reduction fused
      with the (attn_w * h) @ Wo.T contraction (key-time index == hidden index
      since L == H).
Weights/activations feeding matmuls are bf16 (FWL weight loads); state,
gates and softmax math are fp32. Measured model error ~3e-3 rel.
"""

import numpy as np
import ml_dtypes
from contextlib import ExitStack

import concourse.bass as bass
import concourse.tile as tile
from concourse import bacc, mybir
from concourse.bass import ds
from concourse.bass_utils import run_bass_kernel_spmd

F32 = mybir.dt.float32
BF16 = mybir.dt.bfloat16
AF = mybir.ActivationFunctionType
AX = mybir.AxisListType
BF16NP = ml_dtypes.bfloat16

E, H, L_FULL, B, NH, HD = 256, 512, 512, 32, 8, 64
G = 4 * H
P = 128
NCORES = 8
BPC = B // NCORES
KCH = H // P   # 4
MCH = G // P   # 16
ECH = E // P   # 2
U = 8          # recurrence steps per gin block; 2 blocks per For_i body


def build_program(L=L_FULL, bpc=BPC, n_devices=NCORES):
    nc = bacc.Bacc("TRN2", target_bir_lowering=False, debug=False,
                   num_devices=n_devices)
    TCH = L // P
    assert L % (2 * U) == 0 and L % P == 0

    def din(name, shape, dt):
        return nc.dram_tensor(name, shape, dt, kind="ExternalInput").ap()

    xT = din("xT", [P, ECH, L, bpc], BF16)
    Wih0T = din("Wih0T", [P, ECH, MCH, P], BF16)
    Whh0T = din("Whh0T", [P, KCH, MCH, P], BF16)
    Wih1T = din("Wih1T", [P, KCH, MCH, P], BF16)
    Whh1T = din("Whh1T", [P, KCH, MCH, P], BF16)
    WqT = din("WqT", [P, KCH, KCH, P], BF16)
    WkT = din("WkT", [P, KCH, KCH, P], BF16)
    WoD = din("WoD", [P, KCH, P], BF16)
    b0 = din("b0", [P, MCH], F32)
    b1 = din("b1", [P, MCH], F32)
    bq = din("bq", [P, KCH], F32)
    bk = din("bk", [P, KCH], F32)
    bo = din("bo", [P, 1], F32)
    tril = din("tril", [P, P], F32)
    out = nc.dram_tensor("out", [bpc, L, 1], F32, kind="ExternalOutput").ap()
    g0buf = nc.dram_tensor("g0buf", [MCH, P, L, bpc], F32).ap()
    g1buf = nc.dram_tensor("g1buf", [MCH, P, L, bpc], F32).ap()

    with tile.TileContext(nc) as tc, ExitStack() as ctx:
        persist = ctx.enter_context(tc.tile_pool(name="persist", bufs=1))
        wk = ctx.enter_context(tc.tile_pool(name="wk", bufs=3))
        big = ctx.enter_context(tc.tile_pool(name="big", bufs=2))
        pj = ctx.enter_context(tc.tile_pool(name="pj", bufs=2))
        ps_pool = ctx.enter_context(tc.tile_pool(name="ps", bufs=2, space="PSUM"))
        ps_sc = ctx.enter_context(tc.tile_pool(name="ps_sc", bufs=2, space="PSUM"))

        def load_const(ap_in, shape, dt, tag):
            t = persist.tile(shape, dt, tag=tag)
            nc.sync.dma_start(out=t[:], in_=ap_in)
            return t

        sxT = load_const(xT, [P, ECH, L, bpc], BF16, "sxT")
        sWih0 = load_const(Wih0T, [P, ECH, MCH, P], BF16, "sWih0")
        sWhh0 = load_const(Whh0T, [P, KCH, MCH, P], BF16, "sWhh0")
        sWih1 = load_const(Wih1T, [P, KCH, MCH, P], BF16, "sWih1")
        sWhh1 = load_const(Whh1T, [P, KCH, MCH, P], BF16, "sWhh1")
        sWqT = load_const(WqT, [P, KCH, KCH, P], BF16, "sWqT")
        sWkT = load_const(WkT, [P, KCH, KCH, P], BF16, "sWkT")
        sWoD = load_const(WoD, [P, KCH, P], BF16, "sWoD")
        sb0 = load_const(b0, [P, MCH], F32, "sb0")
        sb1 = load_const(b1, [P, MCH], F32, "sb1")
        sbq = load_const(bq, [P, KCH], F32, "sbq")
        sbk = load_const(bk, [P, KCH], F32, "sbk")
        sbo = load_const(bo, [P, 1], F32, "sbo")
        stril = load_const(tril, [P, P], F32, "stril")

        hT0 = persist.tile([P, KCH, L, bpc], BF16, tag="hT0")
        hT1 = persist.tile([P, KCH, L, bpc], BF16, tag="hT1")

        # ---------- input projections ----------
        def proj_to_gbuf(Wsb, kch, rhs_fn, bias_sb, gbuf):
            ncols = L * bpc
            CB = min(512, ncols)
            tpb = CB // bpc
            for m in range(MCH):
                for n in range(ncols // CB):
                    ps = ps_pool.tile([P, CB], F32, tag="ps_mm")
                    for k in range(kch):
                        nc.tensor.matmul(ps[:], Wsb[:, k, m, :], rhs_fn(k, n, tpb),
                                         start=(k == 0), stop=(k == kch - 1))
                    sb = pj.tile([P, CB], F32, tag="sb_proj")
                    nc.vector.tensor_scalar_add(sb[:], ps[:],
                                                bias_sb[:, m:m + 1])
                    nc.sync.dma_start(
                        out=gbuf[m, :, n * tpb:(n + 1) * tpb, :],
                        in_=sb[:].rearrange("p (t b) -> p t b", b=bpc))

        proj_to_gbuf(
            sWih0, ECH,
            lambda k, n, tpb: sxT[:, k, n * tpb:(n + 1) * tpb, :]
            .rearrange("p t b -> p (t b)"),
            sb0, g0buf)

        # ---------- recurrence ----------
        def recurrence(Wsb, gbuf, hT, li):
            c_st = persist.tile([P, KCH, bpc], F32, tag=f"c{li}")
            h_st = persist.tile([P, KCH, 2, bpc], BF16, tag=f"hst{li}")
            nc.vector.memset(c_st[:], 0.0)
            nc.vector.memset(h_st[:], 0.0)
            gin = [persist.tile([P, MCH, U, bpc], F32, tag=f"gin{j}_{li}",
                                name=f"gin{j}_{li}")
                   for j in (0, 1)]
            with tc.For_i(0, L, 2 * U) as t0:
                for j in range(2):
                    nc.sync.dma_start(
                        out=gin[j][:],
                        in_=gbuf[:, :, ds(t0 + j * U, U), :]
                        .rearrange("m p t b -> p m t b"))
                    for u in range(U):
                        s_idx = j * U + u
                        rd_sl = s_idx % 2
                        wr_sl = 1 - rd_sl
                        ps = ps_pool.tile([P, MCH, bpc], F32, tag="ps_mm")
                        for m in range(MCH):
                            for k in range(KCH):
                                nc.tensor.matmul(ps[:, m, :], Wsb[:, k, m, :],
                                                 h_st[:, k, rd_sl, :],
                                                 start=(k == 0),
                                                 stop=(k == KCH - 1))
                        gf = wk.tile([P, MCH, bpc], F32, tag="gf")
                        nc.vector.tensor_add(gf[:], ps[:], gin[j][:, :, u, :])
                        sg = wk.tile([P, 12, bpc], F32, tag="sg")
                        nc.scalar.activation(sg[:], gf[:, 0:12, :], AF.Sigmoid)
                        tg = wk.tile([P, KCH, bpc], F32, tag="tg")
                        nc.scalar.activation(tg[:], gf[:, 12:16, :], AF.Tanh)
                        t1 = wk.tile([P, KCH, bpc], F32, tag="t1")
                        nc.vector.tensor_mul(t1[:], sg[:, 0:4, :], tg[:])
                        t2 = wk.tile([P, KCH, bpc], F32, tag="t2")
                        nc.vector.tensor_mul(t2[:], sg[:, 4:8, :], c_st[:])
                        nc.vector.tensor_add(c_st[:], t1[:], t2[:])
                        tch = wk.tile([P, KCH, bpc], F32, tag="tch")
                        nc.scalar.activation(tch[:], c_st[:], AF.Tanh)
                        nc.vector.tensor_mul(h_st[:, :, wr_sl, :],
                                             sg[:, 8:12, :], tch[:])
                        nc.gpsimd.tensor_copy(
                            hT[:, :, ds(t0 + s_idx, 1), :]
                            .rearrange("p k o b -> p k (o b)"),
                            h_st[:, :, wr_sl, :])

        recurrence(sWhh0, g0buf, hT0, 0)

        proj_to_gbuf(
            sWih1, KCH,
            lambda k, n, tpb: hT0[:, k, n * tpb:(n + 1) * tpb, :]
            .rearrange("p t b -> p (t b)"),
            sb1, g1buf)

        recurrence(sWhh1, g1buf, hT1, 1)

        # ---------- attention + output ----------
        for s in range(bpc):
            qT = persist.tile([P, KCH, L], BF16, tag="qTs")
            kT = persist.tile([P, KCH, L], BF16, tag="kTs")
            for (Wp, bvec, dst, tg_) in ((sWqT, sbq, qT, "ps_qk"),
                                         (sWkT, sbk, kT, "ps_qk")):
                for m in range(KCH):
                    psq = ps_pool.tile([P, L], F32, tag=tg_)
                    for k in range(KCH):
                        nc.tensor.matmul(psq[:], Wp[:, k, m, :],
                                         hT1[:, k, :, s],
                                         start=(k == 0), stop=(k == KCH - 1))
                    nc.vector.tensor_scalar_add(dst[:, m, :], psq[:],
                                                bvec[:, m:m + 1])

            # hw[t_part, tch, hid] = (h1.T)^T scaled by Wo  (per sample)
            hw = persist.tile([P, TCH, H], F32, tag="hw")
            for r in range(KCH):
                for c in range(TCH):
                    pst = ps_pool.tile([P, P], F32, tag="ps_qk")
                    nc.tensor.matmul(pst[:], hT1[:, r, c * P:(c + 1) * P, s],
                                     sWoD[:, r, :], start=True, stop=True)
                    nc.vector.tensor_copy(hw[:, c, r * P:(r + 1) * P], pst[:])

            for qt in range(TCH):
                ncols = (qt + 1) * P
                nacc = wk.tile([P, NH], F32, tag="nacc")
                dacc = wk.tile([P, NH], F32, tag="dacc")
                for hp in range(NH // 2):
                    pss = ps_sc.tile([P, 2, 512], F32, tag="ps_s")
                    for hh in range(2):
                        nc.tensor.matmul(
                            pss[:, hh, 0:ncols],
                            qT[hh * 64:(hh + 1) * 64, hp, qt * P:(qt + 1) * P],
                            kT[hh * 64:(hh + 1) * 64, hp, 0:ncols],
                            start=True, stop=True)
                    Ee = big.tile([P, 2, 512], F32, tag="Ee")
                    nc.scalar.activation(Ee[:, :, 0:ncols], pss[:, :, 0:ncols],
                                         AF.Exp, scale=0.125)
                    for hh in range(2):
                        h_idx = 2 * hp + hh
                        nc.vector.tensor_mul(Ee[:, hh, qt * P:ncols],
                                             Ee[:, hh, qt * P:ncols], stril[:])
                        Em = big.tile([P, 512], F32, tag="Em")
                        nc.vector.tensor_mul(Em[:, 0:ncols], Ee[:, hh, 0:ncols],
                                             hw[:, qt, 0:ncols])
                        nc.vector.reduce_sum(nacc[:, h_idx:h_idx + 1],
                                             Em[:, 0:ncols], axis=AX.X)
                        nc.vector.reduce_sum(dacc[:, h_idx:h_idx + 1],
                                             Ee[:, hh, 0:ncols], axis=AX.X)
                rd = wk.tile([P, NH], F32, tag="rdt")
                nc.vector.reciprocal(rd[:], dacc[:])
                pr = wk.tile([P, NH], F32, tag="pr")
                nc.vector.tensor_mul(pr[:], nacc[:], rd[:])
                osum = wk.tile([P, 1], F32, tag="osum")
                nc.vector.reduce_sum(osum[:], pr[:], axis=AX.X)
                oo = wk.tile([P, 1], F32, tag="oo")
                nc.vector.tensor_scalar(oo[:], osum[:], 0.125, sbo[:, 0:1],
                                        op0=mybir.AluOpType.mult,
                                        op1=mybir.AluOpType.add)
                nc.sync.dma_start(out=out[s, qt * P:(qt + 1) * P, :], in_=oo[:])

    nc.compile()
    return nc


def _reorder_rows(W):
    # gate order i,f,g,o -> i,f,o,g so sigmoid block is contiguous
    return np.concatenate([W[0:H], W[H:2 * H], W[3 * H:4 * H], W[2 * H:3 * H]], 0)


def _wT_layout(Wr, kch):
    # [G, K] -> lhsT tiles [P, kch, MCH, P]
    return np.ascontiguousarray(
        Wr.T.reshape(kch, P, MCH, P).transpose(1, 0, 2, 3))


def prep_shared_inputs(inputs, L=L_FULL):
    f = {}
    f["Wih0T"] = _wT_layout(_reorder_rows(inputs["Wih0"]), ECH).astype(BF16NP)
    f["Whh0T"] = _wT_layout(_reorder_rows(inputs["Whh0"]), KCH).astype(BF16NP)
    f["Wih1T"] = _wT_layout(_reorder_rows(inputs["Wih1"]), KCH).astype(BF16NP)
    f["Whh1T"] = _wT_layout(_reorder_rows(inputs["Whh1"]), KCH).astype(BF16NP)
    f["WqT"] = np.ascontiguousarray(
        inputs["Wq"].T.reshape(KCH, P, KCH, P).transpose(1, 0, 2, 3)).astype(BF16NP)
    f["WkT"] = np.ascontiguousarray(
        inputs["Wk"].T.reshape(KCH, P, KCH, P).transpose(1, 0, 2, 3)).astype(BF16NP)
    wod = np.zeros((P, KCH, P), np.float32)
    for r in range(KCH):
        wod[:, r, :] = np.diag(inputs["Wo"][0, r * P:(r + 1) * P])
    f["WoD"] = wod.astype(BF16NP)
    b0r = _reorder_rows((inputs["bih0"] + inputs["bhh0"]).reshape(4 * H, 1))[:, 0]
    b1r = _reorder_rows((inputs["bih1"] + inputs["bhh1"]).reshape(4 * H, 1))[:, 0]
    f["b0"] = np.ascontiguousarray(b0r.reshape(MCH, P).T).astype(np.float32)
    f["b1"] = np.ascontiguousarray(b1r.reshape(MCH, P).T).astype(np.float32)
    f["bq"] = np.ascontiguousarray(
        inputs["bq"].reshape(KCH, P).T).astype(np.float32)
    f["bk"] = np.ascontiguousarray(
        inputs["bk"].reshape(KCH, P).T).astype(np.float32)
    f["bo"] = np.full((P, 1), np.float32(inputs["bo"][0]), np.float32)
    f["tril"] = np.tril(np.ones((P, P), np.float32))
    return f


def prep_xT(x_slice, L, bpc):
    # [bpc, L, E] -> [P, ECH, L, bpc]
    return np.ascontiguousarray(
        x_slice.transpose(2, 1, 0).reshape(ECH, P, L, bpc)
        .transpose(1, 0, 2, 3)).astype(BF16NP)


_CACHE = {}


def kernel(**inputs):
    inputs = {k: np.asarray(v) for k, v in inputs.items()}
    if "nc" not in _CACHE:
        _CACHE["nc"] = build_program()
    nc = _CACHE["nc"]
    shared = prep_shared_inputs(inputs)
    in_maps = []
    for c in range(NCORES):
        m = dict(shared)
        m["xT"] = prep_xT(inputs["x"][c * BPC:(c + 1) * BPC], L_FULL, BPC)
        in_maps.append(m)
    res = run_bass_kernel_spmd(nc, in_maps, core_ids=list(range(NCORES)))
    out = np.concatenate([res.results[c]["out"] for c in range(NCORES)], 0)
    return out.astype(np.float32)


if __name__ == "__main__":
    import sys
    sys.path.insert(0, "/root/problem")
    np.random.seed(0)
    import reference as R
    inp = {k: np.asarray(v) for k, v in R.setup_inputs().items()}
    got = kernel(**inp)
    print("kernel out shape:", got.shape, got.dtype)
